# revision 1
# baseline (speedup 1.0000x reference)
"""Trainium2 Bass kernel for nn_Bilinear_54065048322517.

Math:  out[b, j] = input2[b, j] * sum_{i,k} weights[i, j, k] * input1[b, i]
           =   input2 * (input1 @ weights.sum(axis=2))
Shapes: input1 (16384, 64) f32, input2 (16384, 2048) f32,
        weights (64, 2048, 64) f32, out (16384, 2048) f32.

Sharding: split J=2048 into 8 shards of 256 (one per NeuronCore).
Each core reads: input1 full (4MB) + its input2 shard (16MB) + its
weights shard (4MB), writes its out shard (16MB) -> 40MB of HBM
traffic per core (vs 64.5MB for batch sharding, which would have to
replicate the 32MB weights).

Built on bacc.Bacc + TileContext; Bacc.compile() legalizes the
one-embedded-wait-per-instruction TRN2 constraint by splitting extra
waits into event-semaphore instructions.

Per-core kernel (all fp32):
  phase A (interleaved with phase B groups):
    - weights shard loaded as 8 chunks (128, 1024) on the ACT ring
      with partition 2i+h (h = j-half; (i,h) strides merge to a
      uniform 8192-elem partition stride); two DVE grouped-reduces
      over K -> w2tmp (128, 128); two permutation-matrix matmuls
      P_h.T @ w2tmp (P_h[2i+h, i] = P_h[2i+h, 64+i] = 1) + ACT
      copies de-interleave/duplicate into w2dup (128, 256) where
      partition q*64+i holds w2[i, :] for both q.
    - input1 loaded as 8 chunks (128, 1024) on the SP ring,
      interleaved with the first x2 group loads: partition p holds
      rows {256n + 2p + q} (512B contiguous runs); 64 TensorE
      (128,128) transposes -> x1T (128, 8192):
      x1T[q*64 + i, n*128 + p] = x1[256n + 2p + q, i].
  phase B (groups of 8 super-tiles = 2048 rows), group g processed
  right after transpose batches 2g+2, 2g+3 so PE streams seamlessly:
    - DMA x2 group -> xtile (128, 4096), SP ring (2KB runs)
    - per super-tile n: 2 matmuls (K=64, M=128, N=256) on DISJOINT
      PE row-groups (q=0 rows 0-63, q=1 rows 64-127, concurrent)
      into separate PSUM banks:
        pt[:, q*512:q*512+256] =
            x1T[64q:64q+64, n*128:+128].T @ w2dup[64q:64q+64]
    - DVE: otile = pt * xtile (strided view over the two banks)
    - DMA otile -> out group in two 1MB halves, SP ring.
"""

import numpy as np

B, I, J, K = 16384, 64, 2048, 64
NCORES = 8
JS = J // NCORES          # 256 columns per core
NSUP = B // 256           # 64 super-tiles of 256 rows
GROUP = 8                 # super-tiles per DMA group
NG = NSUP // GROUP        # 8 groups
NWCHUNK = 8               # weights load chunks (all resident)
NXCHUNK = 8               # input1 load chunks
XBUFS = 3                 # xtile buffer depth
OBUFS = 2                 # otile buffer depth

_CACHE = {}


def _build_nc():
    from contextlib import ExitStack

    import concourse.mybir as mybir
    import concourse.tile as tile
    from concourse import bacc, masks

    f32 = mybir.dt.float32
    nc = bacc.Bacc()

    x1 = nc.dram_tensor("input1", [B, I], f32, kind="ExternalInput")
    x2 = nc.dram_tensor("input2", [B, JS], f32, kind="ExternalInput")
    w = nc.dram_tensor("weights", [I, JS, K], f32, kind="ExternalInput")
    out = nc.dram_tensor("out", [B, JS], f32, kind="ExternalOutput")

    with tile.TileContext(nc) as tc, ExitStack() as ctx:
        const_pool = ctx.enter_context(tc.tile_pool(name="const", bufs=1))
        stage_pool = ctx.enter_context(tc.tile_pool(name="stage", bufs=1))
        wc_pool = ctx.enter_context(tc.tile_pool(name="wc", bufs=1))
        x_pool = ctx.enter_context(tc.tile_pool(name="xin", bufs=XBUFS))
        o_pool = ctx.enter_context(tc.tile_pool(name="oout", bufs=OBUFS))
        ps_pool = ctx.enter_context(tc.tile_pool(name="ps", bufs=3, space="PSUM"))
        tr_pool = ctx.enter_context(tc.tile_pool(name="tr", bufs=2, space="PSUM"))

        identity = const_pool.tile([128, 128], f32)
        masks.make_identity(nc, identity[:])

        # permutation masks: P[h][2i+h, i] = P[h][2i+h, 64+i] = 1, else 0
        # (P_h.T @ w2tmp)[q*64+i, j''] = w2tmp[2i+h, j'']
        perm = []
        for h in range(2):
            ph = const_pool.tile([128, 128], f32, name=f"perm{h}")
            perm.append(ph)
            nc.gpsimd.memset(ph[:], 0.0)
            for q in range(2):
                # select p - 2*m - h == 0 over the (128, 64) column block
                nc.gpsimd.affine_select(
                    out=ph[:, q * 64 : (q + 1) * 64],
                    in_=ph[:, q * 64 : (q + 1) * 64],
                    compare_op=mybir.AluOpType.not_equal,
                    fill=1.0,
                    base=-h,
                    pattern=[[-2, 64]],
                    channel_multiplier=1,
                )

        # ---- input1 chunk loads (SP ring) + x2 prefetch interleave ----
        x1stage = stage_pool.tile([128, B * I // 128], f32)  # (128, 8192)
        x1_r = x1.rearrange("(n p q) i -> p n q i", p=128, q=2)  # (128,64,2,64)
        xcsz = B * I // 128 // NXCHUNK  # 1024 elems/partition per chunk
        xnsz = NSUP // NXCHUNK          # 8 super-tiles per chunk

        def load_x1_chunk(k):
            nc.sync.dma_start(
                out=x1stage[:, k * xcsz : (k + 1) * xcsz].rearrange(
                    "p (n q i) -> p n q i", q=2, i=I
                ),
                in_=x1_r[:, k * xnsz : (k + 1) * xnsz],
            )

        x2_r = x2.rearrange(
            "(g s p q) j -> g p s q j", g=NG, s=GROUP, p=128, q=2
        )
        out_r = out.rearrange(
            "(g sh s p q) j -> g sh p s q j", g=NG, sh=2, s=GROUP // 2, p=128, q=2
        )

        xtiles = []

        def load(g):
            assert len(xtiles) == g
            xt = x_pool.tile([128, GROUP * 2 * JS], f32, name=f"xt{g}", tag="xt")
            xtiles.append(xt)
            nc.sync.dma_start(
                out=xt[:].rearrange("p (s q j) -> p s q j", s=GROUP, q=2),
                in_=x2_r[g],
            )

        for k in range(4):
            load_x1_chunk(k)
        load(0)
        for k in range(4, NXCHUNK):
            load_x1_chunk(k)
        load(1)
        load(2)

        # ---- weights load (ACT ring, 128 partitions) + K-reduction ----
        # chunk c: partition 2i+h <- W[i, 128h + c*16 + j'', :], 4KB runs
        w_v = w.rearrange("i (h c j) k -> c i h (j k)", h=2, c=NWCHUNK)
        w2tmp = const_pool.tile([128, JS // 2], f32)  # (128, 128), part 2i+h
        csz = JS * K // NWCHUNK // 2  # 1024 elems per partition per chunk
        jcs = JS // NWCHUNK // 2      # 16 w2tmp columns per chunk
        wchunks = []
        for c in range(NWCHUNK):
            wchunk = wc_pool.tile(
                [128, csz], f32, name=f"wchunk{c}", tag=f"wchunk{c}"
            )
            wchunks.append(wchunk)
            nc.scalar.dma_start(
                out=wchunk[:],
                in_=w_v[c].rearrange("i h f -> (i h) f"),
            )
        # two big reduces (fewer DVE drains); chunks are column-adjacent
        # in w2tmp and contiguous SBUF slots are NOT guaranteed, so reduce
        # each chunk's 16-column slice but batch 4 per instruction via AP?
        # -> keep it simple: one reduce per chunk-pair is not possible
        #    across tiles; do per-chunk reduces into w2tmp.
        for c in range(NWCHUNK):
            nc.vector.tensor_reduce(
                out=w2tmp[:, c * jcs : (c + 1) * jcs],
                in_=wchunks[c][:].rearrange("p (j k) -> p j k", k=K),
                axis=mybir.AxisListType.X,
                op=mybir.AluOpType.add,
            )

        # de-interleave + duplicate via permutation matmuls:
        # w2dup[q*64+i, 128h+j''] = w2tmp[2i+h, j'']
        w2dup = const_pool.tile([128, JS], f32)
        for h in range(2):
            pw = tr_pool.tile([128, 512], f32, name="tt", tag="tt")
            nc.tensor.matmul(
                pw[:, 0:128], lhsT=perm[h][:], rhs=w2tmp[:],
                start=True, stop=True,
            )
            nc.scalar.copy(w2dup[:, h * 128 : (h + 1) * 128], pw[:, 0:128])

        # ---- transposes + groups, interleaved ----
        x1T = const_pool.tile([128, NSUP * 128], f32)  # (128, 8192)

        def transpose_batch(m):
            tt = tr_pool.tile([128, 512], f32, name="tt", tag="tt")
            for s in range(4):
                n = m * 4 + s
                nc.tensor.transpose(
                    tt[:, s * 128 : (s + 1) * 128],
                    x1stage[:, n * 128 : (n + 1) * 128],
                    identity[:],
                )
            nc.scalar.copy(x1T[:, m * 512 : (m + 1) * 512], tt[:])

        def process(g):
            xtile = xtiles[g]
            ot = o_pool.tile([128, GROUP * 2 * JS], f32, name=f"ot{g}", tag="ot")
            for s in range(GROUP):
                n = g * GROUP + s
                # 2 banks: each concurrent row-group matmul drains into its
                # own PSUM bank (q=0 -> cols 0:256, q=1 -> cols 512:768)
                pt = ps_pool.tile([128, 4 * JS], f32)  # (128, 1024)
                for q in range(2):
                    nc.tensor.matmul(
                        pt[:, q * 2 * JS : q * 2 * JS + JS],
                        lhsT=x1T[q * 64 : (q + 1) * 64, n * 128 : (n + 1) * 128],
                        rhs=w2dup[q * 64 : (q + 1) * 64, :],
                        start=True,
                        stop=True,
                    )
                nc.vector.tensor_mul(
                    ot[:, s * 512 : (s + 1) * 512].rearrange(
                        "p (q j) -> p q j", q=2
                    ),
                    pt[:].rearrange("p (q j) -> p q j", q=2)[:, :, 0:JS],
                    xtile[:, s * 512 : (s + 1) * 512].rearrange(
                        "p (q j) -> p q j", q=2
                    ),
                )
                if s == GROUP // 2 - 1 or s == GROUP - 1:
                    sh = 0 if s < GROUP // 2 else 1
                    nc.sync.dma_start(
                        out=out_r[g, sh],
                        in_=ot[
                            :, sh * GROUP * JS : (sh + 1) * GROUP * JS
                        ].rearrange("p (s q j) -> p s q j", s=GROUP // 2, q=2),
                    )
            if g + XBUFS < NG:
                load(g + XBUFS)

        for g in range(NG):
            transpose_batch(2 * g)
            transpose_batch(2 * g + 1)
            if g >= 1:
                process(g - 1)
        process(NG - 1)

    nc.compile()
    return nc


def _get_nc():
    if "nc" not in _CACHE:
        _CACHE["nc"] = _build_nc()
    return _CACHE["nc"]


def _make_in_maps(input1, input2, weights):
    input1 = np.ascontiguousarray(input1, dtype=np.float32)
    in_maps = []
    for c in range(NCORES):
        sl = slice(c * JS, (c + 1) * JS)
        in_maps.append(
            {
                "input1": input1,
                "input2": np.ascontiguousarray(input2[:, sl], dtype=np.float32),
                "weights": np.ascontiguousarray(weights[:, sl, :], dtype=np.float32),
            }
        )
    return in_maps


def run(input1, input2, weights, trace=False, **spmd_kwargs):
    from concourse.bass_utils import run_bass_kernel_spmd

    nc = _get_nc()
    in_maps = _make_in_maps(input1, input2, weights)
    res = run_bass_kernel_spmd(
        nc, in_maps, core_ids=list(range(NCORES)), trace=trace, **spmd_kwargs
    )
    outs = [res.results[c]["out"] for c in range(NCORES)]
    full = np.concatenate(outs, axis=1)
    return full, res


def kernel(input1, input2, weights):
    full, _ = run(input1, input2, weights, trace=False)
    return full



# revision 2
# speedup vs baseline: 1.8891x; 1.8891x over previous
"""Trainium2 Bass kernel for nn_Bilinear_54065048322517.

Math:  out[b, j] = input2[b, j] * sum_{i,k} weights[i, j, k] * input1[b, i]
           =   input2 * (input1 @ weights.sum(axis=2))
Shapes: input1 (16384, 64) f32, input2 (16384, 2048) f32,
        weights (64, 2048, 64) f32, out (16384, 2048) f32.

Sharding: split J=2048 into 8 shards of 256 (one per NeuronCore).
J-sharding avoids replicating the 32MB weights tensor (batch sharding
would need 64.5MB of HBM traffic per core vs 40MB f32 here).

Precision: the whole pipeline runs in bf16 (HBM side), halving traffic
to 20MB per core: input1 2MB + input2 shard 8MB + weights shard 2MB
read, out shard 8MB written.  The K-reduction of weights and the GEMM
accumulate in f32 (DVE reduce output f32, matmul PSUM f32); bf16
rounding enters only on the stored operands and the final output
(rel-l2 vs the f32 reference ~4e-3, well under the 2e-2 gate).  The
host casts/gathers around the kernel call.

Host-side staging puts every per-core DRAM array in the exact tile
layout the kernel consumes, so all DMAs move 8-32KB contiguous runs
per partition:
  x1t[q*64+i, n*128+p]              = input1[n*256 + 2p + q, i]
  wd[h*64+i, j''*64+k]              = weights[i, jsl(h*128+j''), k]
  x2d[g, p, ((s*2+q)*256+j)]        = input2[(g*8+s)*256 + 2p + q, jsl(j)]
  (out is produced in the x2d layout and un-permuted on the host)

Per-core kernel:
  - wd loaded in 2 chunks (scalar/ACT ring + sync/SP ring, 1MB each);
    DVE tensor_reduce over k -> w2h (128, 128) f32 where partition
    h*64+i holds w2[i, h-half columns].
  - two selection-matrix matmuls (sel_h[h*64+i, q*64+i] = 1) rearrange
    w2h into pdup[q*64+i, h*128+j''] = w2[i, j'] (PSUM f32), and one
    DVE copy casts it to w2dup (128, 256) bf16: each PE row-group
    q holds the full (64, 256) reduced weight matrix.
  - x1t loaded on the sync ring (0.5MB head chunk + 1.5MB tail).
  - 8 groups of 8 super-tiles (256 rows each): per group, one 1MB x2
    load (scalar ring); per super-tile two concurrent matmuls on
    disjoint PE row-groups (q=0 rows 0-63, q=1 rows 64-127) into
    separate PSUM banks, then DVE  otile = psum * xtile  (bf16 out);
    one 1MB store per group (sync ring).
"""

import numpy as np

B, I, J, K = 16384, 64, 2048, 64
NCORES = 8
JS = J // NCORES          # 256 columns per core
NSUP = B // 256           # 64 super-tiles of 256 rows
GROUP = 8                 # super-tiles per DMA group
NG = NSUP // GROUP        # 8 groups
XBUFS = 4                 # xtile buffer depth
OBUFS = 3                 # otile buffer depth
GFREE = GROUP * 2 * JS    # 4096 elems per partition per group

_CACHE = {}


def _build_nc():
    from contextlib import ExitStack

    import concourse.mybir as mybir
    import concourse.tile as tile
    from concourse import bacc

    f32 = mybir.dt.float32
    bf16 = mybir.dt.bfloat16
    nc = bacc.Bacc()

    x1 = nc.dram_tensor("input1", [128, NSUP * 128], bf16, kind="ExternalInput")
    x2 = nc.dram_tensor("input2", [NG, 128, GFREE], bf16, kind="ExternalInput")
    w = nc.dram_tensor("weights", [128, (JS // 2) * K], bf16, kind="ExternalInput")
    out = nc.dram_tensor("out", [NG, 128, GFREE], bf16, kind="ExternalOutput")

    with tile.TileContext(nc) as tc, ExitStack() as ctx:
        const_pool = ctx.enter_context(tc.tile_pool(name="const", bufs=1))
        wc_pool = ctx.enter_context(tc.tile_pool(name="wc", bufs=1))
        x_pool = ctx.enter_context(tc.tile_pool(name="xin", bufs=XBUFS))
        o_pool = ctx.enter_context(tc.tile_pool(name="oout", bufs=OBUFS))
        ps_pool = ctx.enter_context(tc.tile_pool(name="ps", bufs=3, space="PSUM"))
        tr_pool = ctx.enter_context(tc.tile_pool(name="tr", bufs=1, space="PSUM"))

        # selection masks: sel_h[h*64+i, q*64+i] = 1, else 0
        # (sel_h.T @ w2h)[q*64+i, j''] = w2h[h*64+i, j'']
        sel = []
        for h in range(2):
            sh = const_pool.tile([128, 128], f32, name=f"sel{h}")
            sel.append(sh)
            nc.gpsimd.memset(sh[:], 0.0)
            for q in range(2):
                # fill where p - 64h - m' == 0 over the (128, 64) column block
                nc.gpsimd.affine_select(
                    out=sh[:, q * 64 : (q + 1) * 64],
                    in_=sh[:, q * 64 : (q + 1) * 64],
                    compare_op=mybir.AluOpType.not_equal,
                    fill=1.0,
                    base=-64 * h,
                    pattern=[[-1, 64]],
                    channel_multiplier=1,
                )

        # ---- weights: 2 chunk loads (one per HWDGE ring) + K-reduce ----
        w2h = const_pool.tile([128, JS // 2], f32)  # (128, 128), part h*64+i
        wcsz = (JS // 2) * K // 2  # 4096 elems per partition per chunk
        for c in range(2):
            wchunk = wc_pool.tile([128, wcsz], bf16, name=f"wchunk{c}", tag=f"wc{c}")
            eng = nc.scalar if c == 0 else nc.sync
            eng.dma_start(
                out=wchunk[:], in_=w[:, c * wcsz : (c + 1) * wcsz]
            )
            nc.vector.tensor_reduce(
                out=w2h[:, c * 64 : (c + 1) * 64],
                in_=wchunk[:].rearrange("p (j k) -> p j k", k=K),
                axis=mybir.AxisListType.X,
                op=mybir.AluOpType.add,
            )

        # ---- x1t load on sync ring (head chunk first for group 0/1) ----
        x1T = const_pool.tile([128, NSUP * 128], bf16)
        nc.sync.dma_start(out=x1T[:, 0:2048], in_=x1[:, 0:2048])
        nc.sync.dma_start(out=x1T[:, 2048:], in_=x1[:, 2048:])

        # ---- x2 group prefetch (scalar ring) ----
        xtiles = []

        def load(g):
            assert len(xtiles) == g
            xt = x_pool.tile([128, GFREE], bf16, name=f"xt{g}", tag="xt")
            xtiles.append(xt)
            nc.scalar.dma_start(out=xt[:], in_=x2[g])

        for g in range(XBUFS):
            load(g)

        # ---- de-interleave + duplicate w2 via selection matmuls ----
        # pdup[q*64+i, h*128+j''] = w2h[h*64+i, j'']
        pdup = tr_pool.tile([128, JS], f32)
        for h in range(2):
            nc.tensor.matmul(
                pdup[:, h * 128 : (h + 1) * 128],
                lhsT=sel[h][:],
                rhs=w2h[:],
                start=True,
                stop=True,
            )
        w2dup = const_pool.tile([128, JS], bf16)
        nc.vector.tensor_copy(out=w2dup[:], in_=pdup[:])

        # ---- main loop ----
        def process(g):
            xt = xtiles[g]
            ot = o_pool.tile([128, GFREE], bf16, name=f"ot{g}", tag="ot")
            for s in range(GROUP):
                n = g * GROUP + s
                # 2 banks: each concurrent row-group matmul drains into its
                # own PSUM bank (q=0 -> cols 0:256, q=1 -> cols 512:768)
                pt = ps_pool.tile([128, 4 * JS], f32)  # (128, 1024)
                for q in range(2):
                    nc.tensor.matmul(
                        pt[:, q * 2 * JS : q * 2 * JS + JS],
                        lhsT=x1T[q * 64 : (q + 1) * 64, n * 128 : (n + 1) * 128],
                        rhs=w2dup[q * 64 : (q + 1) * 64, :],
                        start=True,
                        stop=True,
                    )
                nc.vector.tensor_mul(
                    ot[:, s * 512 : (s + 1) * 512].rearrange(
                        "p (q j) -> p q j", q=2
                    ),
                    pt[:].rearrange("p (q j) -> p q j", q=2)[:, :, 0:JS],
                    xt[:, s * 512 : (s + 1) * 512].rearrange(
                        "p (q j) -> p q j", q=2
                    ),
                )
            nc.sync.dma_start(out=out[g], in_=ot[:])
            if g + XBUFS < NG:
                load(g + XBUFS)

        for g in range(NG):
            process(g)

    nc.compile()
    return nc


def _get_nc():
    if "nc" not in _CACHE:
        _CACHE["nc"] = _build_nc()
    return _CACHE["nc"]


def _make_in_maps(input1, input2, weights):
    import ml_dtypes

    BF = ml_dtypes.bfloat16
    input1 = np.asarray(input1, dtype=np.float32)
    input2 = np.asarray(input2, dtype=np.float32)
    weights = np.asarray(weights, dtype=np.float32)

    # x1t[q*64+i, n*128+p] = input1[n*256 + 2p + q, i]
    x1t = (
        input1.reshape(NSUP, 128, 2, I)
        .transpose(2, 3, 0, 1)
        .reshape(128, NSUP * 128)
        .astype(BF)
    )

    in_maps = []
    for c in range(NCORES):
        sl = slice(c * JS, (c + 1) * JS)
        # wd[h*64+i, j''*64+k] = weights[i, c*JS + h*128 + j'', k]
        wd = (
            weights[:, sl, :]
            .reshape(I, 2, 128, K)
            .transpose(1, 0, 2, 3)
            .reshape(128, 128 * K)
            .astype(BF)
        )
        # x2d[g, p, ((s*2+q)*256+j)] = input2[(g*8+s)*256 + 2p + q, sl][j]
        x2d = (
            input2[:, sl]
            .reshape(NG, GROUP, 128, 2, JS)
            .transpose(0, 2, 1, 3, 4)
            .reshape(NG, 128, GFREE)
            .astype(BF)
        )
        in_maps.append({"input1": x1t, "input2": x2d, "weights": wd})
    return in_maps


def run(input1, input2, weights, trace=False, **spmd_kwargs):
    from concourse.bass_utils import run_bass_kernel_spmd

    nc = _get_nc()
    in_maps = _make_in_maps(input1, input2, weights)
    res = run_bass_kernel_spmd(
        nc, in_maps, core_ids=list(range(NCORES)), trace=trace, **spmd_kwargs
    )
    outs = []
    for c in range(NCORES):
        o = np.asarray(res.results[c]["out"])  # (NG, 128, GFREE) bf16
        outs.append(
            o.reshape(NG, 128, GROUP, 2, JS)
            .transpose(0, 2, 1, 3, 4)
            .reshape(B, JS)
        )
    full = np.concatenate(outs, axis=1).astype(np.float32)
    return full, res


def kernel(input1, input2, weights):
    full, _ = run(input1, input2, weights, trace=False)
    return full


# revision 3
# speedup vs baseline: 1.9069x; 1.0094x over previous
"""Trainium2 Bass kernel for nn_Bilinear_54065048322517.

Math:  out[b, j] = input2[b, j] * sum_{i,k} weights[i, j, k] * input1[b, i]
           =   input2 * (input1 @ weights.sum(axis=2))
Shapes: input1 (16384, 64) f32, input2 (16384, 2048) f32,
        weights (64, 2048, 64) f32, out (16384, 2048) f32.

Sharding: split J=2048 into 8 shards of 256 (one per NeuronCore);
J-sharding avoids replicating the 32MB weights tensor.

Precision: the HBM side runs in bf16, halving traffic to 20MB per
core: input1 2MB + input2 shard 8MB + weights shard 2MB read, out
shard 8MB written.  The K-reduction and GEMM accumulate in f32 (DVE
reduce -> f32, matmul -> PSUM f32); bf16 rounding enters only on the
stored operands and the final output (rel-l2 vs the f32 reference
~4e-3, well under the 2e-2 gate).  The host casts/gathers.

Host staging puts every per-core DRAM array in the exact layout the
kernel consumes, so all DMAs move 2-16KB contiguous runs/partition:
  x1t[q*64+i, n*128+p]       = input1[n*256 + 2p + q, i]
  wd[h*64+i, j''*64+k]       = weights[i, jsl(h*128+j''), k]
  x2d[g, p, (s*2+q)*256+j]   = input2[(g*8+s)*256 + 2p + q, jsl(j)]
  (out is produced in the x2d layout and un-permuted on the host)

Per-core kernel:
  - wd loaded in 8 chunks of 256KB alternating between the two HWDGE
    rings (scalar/ACT gets even chunks, sync/SP odd), each reduced
    over k by DVE as it lands -> w2h (128, 128) f32, partition h*64+i
    holds w2[i, h-half cols].  Pipelining the reduce under the loads
    keeps the w2 chain off the critical path (v1 lost ~12us here).
  - two selection-matrix matmuls (sel_h[h*64+i, q*64+i] = 1) fan
    w2h out to pdup[q*64+i, h*128+j''] = w2[i, j'] (PSUM f32); one
    DVE copy casts to w2dup (128, 256) bf16 so each PE row-group q
    holds the full reduced weight matrix.
  - x1t loads on the sync ring behind the odd w chunks (0.5MB head
    chunk covering groups 0-1 first, then the 1.5MB rest).
  - 8 groups of 8 super-tiles (256 rows each): per group one 1MB x2
    load (scalar ring); per super-tile PAIR four matmuls (q=0/q=1 on
    disjoint PE row-groups run concurrently; the two u-steps share a
    PSUM bank half each: col q*512 + u*256) into a 2-bank PSUM tile,
    then ONE DVE multiply for the pair (1024 elems/partition -- half
    the per-instruction read-write-bubble overhead of per-super-tile
    muls); two 512KB half-group stores (sync ring).
"""

import numpy as np

B, I, J, K = 16384, 64, 2048, 64
NCORES = 8
JS = J // NCORES          # 256 columns per core
NSUP = B // 256           # 64 super-tiles of 256 rows
GROUP = 8                 # super-tiles per DMA group
NG = NSUP // GROUP        # 8 groups
NPAIR = GROUP // 2        # super-tile pairs per group
XBUFS = 4                 # xtile buffer depth
OBUFS = 3                 # otile buffer depth
NWCHUNK = 8               # weights load chunks
GFREE = GROUP * 2 * JS    # 4096 elems per partition per group

_CACHE = {}


def _build_nc():
    from contextlib import ExitStack

    import concourse.mybir as mybir
    import concourse.tile as tile
    from concourse import bacc

    f32 = mybir.dt.float32
    bf16 = mybir.dt.bfloat16
    nc = bacc.Bacc()

    x1 = nc.dram_tensor("input1", [128, NSUP * 128], bf16, kind="ExternalInput")
    x2 = nc.dram_tensor("input2", [NG, 128, GFREE], bf16, kind="ExternalInput")
    w = nc.dram_tensor("weights", [128, (JS // 2) * K], bf16, kind="ExternalInput")
    out = nc.dram_tensor("out", [NG, 128, GFREE], bf16, kind="ExternalOutput")

    with tile.TileContext(nc) as tc, ExitStack() as ctx:
        const_pool = ctx.enter_context(tc.tile_pool(name="const", bufs=1))
        wc_pool = ctx.enter_context(tc.tile_pool(name="wc", bufs=1))
        x_pool = ctx.enter_context(tc.tile_pool(name="xin", bufs=XBUFS))
        o_pool = ctx.enter_context(tc.tile_pool(name="oout", bufs=OBUFS))
        ps_pool = ctx.enter_context(tc.tile_pool(name="ps", bufs=3, space="PSUM"))
        tr_pool = ctx.enter_context(tc.tile_pool(name="tr", bufs=1, space="PSUM"))

        # selection masks: sel_h[h*64+i, q*64+i] = 1, else 0
        # (sel_h.T @ w2h)[q*64+i, j''] = w2h[h*64+i, j'']
        sel = []
        for h in range(2):
            sh = const_pool.tile([128, 128], f32, name=f"sel{h}")
            sel.append(sh)
            nc.gpsimd.memset(sh[:], 0.0)
            for q in range(2):
                # fill where p - 64h - m' == 0 over the (128, 64) column block
                nc.gpsimd.affine_select(
                    out=sh[:, q * 64 : (q + 1) * 64],
                    in_=sh[:, q * 64 : (q + 1) * 64],
                    compare_op=mybir.AluOpType.not_equal,
                    fill=1.0,
                    base=-64 * h,
                    pattern=[[-1, 64]],
                    channel_multiplier=1,
                )

        # ---- weights: 8 chunk loads alternating rings + pipelined reduce ----
        w2h = const_pool.tile([128, JS // 2], f32)  # (128, 128), part h*64+i
        wcsz = (JS // 2) * K // NWCHUNK  # 1024 elems per partition per chunk
        jcs = (JS // 2) // NWCHUNK       # 16 w2h columns per chunk
        for c in range(NWCHUNK):
            wchunk = wc_pool.tile([128, wcsz], bf16, name=f"wchunk{c}", tag=f"wc{c}")
            eng = nc.scalar if c % 2 == 0 else nc.sync
            eng.dma_start(out=wchunk[:], in_=w[:, c * wcsz : (c + 1) * wcsz])
            nc.vector.tensor_reduce(
                out=w2h[:, c * jcs : (c + 1) * jcs],
                in_=wchunk[:].rearrange("p (j k) -> p j k", k=K),
                axis=mybir.AxisListType.X,
                op=mybir.AluOpType.add,
            )

        # ---- x1t load on sync ring (head chunk first for groups 0-1) ----
        x1T = const_pool.tile([128, NSUP * 128], bf16)
        nc.sync.dma_start(out=x1T[:, 0:2048], in_=x1[:, 0:2048])
        nc.sync.dma_start(out=x1T[:, 2048:], in_=x1[:, 2048:])

        # ---- x2 group prefetch (scalar ring) ----
        xtiles = []

        def load(g):
            assert len(xtiles) == g
            xt = x_pool.tile([128, GFREE], bf16, name=f"xt{g}", tag="xt")
            xtiles.append(xt)
            nc.scalar.dma_start(out=xt[:], in_=x2[g])

        for g in range(XBUFS):
            load(g)

        # ---- de-interleave + duplicate w2 via selection matmuls ----
        # pdup[q*64+i, h*128+j''] = w2h[h*64+i, j'']
        pdup = tr_pool.tile([128, JS], f32)
        for h in range(2):
            nc.tensor.matmul(
                pdup[:, h * 128 : (h + 1) * 128],
                lhsT=sel[h][:],
                rhs=w2h[:],
                start=True,
                stop=True,
            )
        w2dup = const_pool.tile([128, JS], bf16)
        nc.vector.tensor_copy(out=w2dup[:], in_=pdup[:])

        # ---- main loop ----
        def process(g):
            xt = xtiles[g]
            ot = o_pool.tile([128, GFREE], bf16, name=f"ot{g}", tag="ot")
            for mm in range(NPAIR):
                # 4 matmuls into a 2-bank PSUM tile: col q*512 + u*256;
                # the q pair runs concurrently on disjoint PE row-groups
                # into different banks, u-steps fill the bank halves.
                pt = ps_pool.tile([128, 4 * JS], f32)  # (128, 1024)
                for u in range(2):
                    n = (g * NPAIR + mm) * 2 + u
                    for q in range(2):
                        nc.tensor.matmul(
                            pt[:, q * 512 + u * JS : q * 512 + (u + 1) * JS],
                            lhsT=x1T[
                                q * 64 : (q + 1) * 64, n * 128 : (n + 1) * 128
                            ],
                            rhs=w2dup[q * 64 : (q + 1) * 64, :],
                            start=True,
                            stop=True,
                        )
                nc.vector.tensor_mul(
                    ot[:, mm * 1024 : (mm + 1) * 1024].rearrange(
                        "p (u q j) -> p q u j", u=2, q=2
                    ),
                    pt[:].rearrange("p (q u j) -> p q u j", q=2, u=2),
                    xt[:, mm * 1024 : (mm + 1) * 1024].rearrange(
                        "p (u q j) -> p q u j", u=2, q=2
                    ),
                )
                if mm % 2 == 1:
                    half = mm // 2
                    nc.sync.dma_start(
                        out=out[g][:, half * 2048 : (half + 1) * 2048],
                        in_=ot[:, half * 2048 : (half + 1) * 2048],
                    )
            if g + XBUFS < NG:
                load(g + XBUFS)

        for g in range(NG):
            process(g)

    nc.compile()
    return nc


def _get_nc():
    if "nc" not in _CACHE:
        _CACHE["nc"] = _build_nc()
    return _CACHE["nc"]


def _make_in_maps(input1, input2, weights):
    import ml_dtypes

    BF = ml_dtypes.bfloat16
    input1 = np.asarray(input1, dtype=np.float32)
    input2 = np.asarray(input2, dtype=np.float32)
    weights = np.asarray(weights, dtype=np.float32)

    # x1t[q*64+i, n*128+p] = input1[n*256 + 2p + q, i]
    x1t = (
        input1.reshape(NSUP, 128, 2, I)
        .transpose(2, 3, 0, 1)
        .reshape(128, NSUP * 128)
        .astype(BF)
    )

    in_maps = []
    for c in range(NCORES):
        sl = slice(c * JS, (c + 1) * JS)
        # wd[h*64+i, j''*64+k] = weights[i, c*JS + h*128 + j'', k]
        wd = (
            weights[:, sl, :]
            .reshape(I, 2, 128, K)
            .transpose(1, 0, 2, 3)
            .reshape(128, 128 * K)
            .astype(BF)
        )
        # x2d[g, p, (s*2+q)*256+j] = input2[(g*8+s)*256 + 2p + q, sl][j]
        x2d = (
            input2[:, sl]
            .reshape(NG, GROUP, 128, 2, JS)
            .transpose(0, 2, 1, 3, 4)
            .reshape(NG, 128, GFREE)
            .astype(BF)
        )
        in_maps.append({"input1": x1t, "input2": x2d, "weights": wd})
    return in_maps


def run(input1, input2, weights, trace=False, **spmd_kwargs):
    from concourse.bass_utils import run_bass_kernel_spmd

    nc = _get_nc()
    in_maps = _make_in_maps(input1, input2, weights)
    res = run_bass_kernel_spmd(
        nc, in_maps, core_ids=list(range(NCORES)), trace=trace, **spmd_kwargs
    )
    outs = []
    for c in range(NCORES):
        o = np.asarray(res.results[c]["out"])  # (NG, 128, GFREE) bf16
        outs.append(
            o.reshape(NG, 128, GROUP, 2, JS)
            .transpose(0, 2, 1, 3, 4)
            .reshape(B, JS)
        )
    full = np.concatenate(outs, axis=1).astype(np.float32)
    return full, res


def kernel(input1, input2, weights):
    full, _ = run(input1, input2, weights, trace=False)
    return full


# revision 6
# speedup vs baseline: 2.0005x; 1.0491x over previous
"""Trainium2 Bass kernel for nn_Bilinear_54065048322517.

Math:  out[b, j] = input2[b, j] * sum_{i,k} weights[i, j, k] * input1[b, i]
           =   input2 * (input1 @ weights.sum(axis=2))
Shapes: input1 (16384, 64) f32, input2 (16384, 2048) f32,
        weights (64, 2048, 64) f32, out (16384, 2048) f32.

Sharding: split J=2048 into 8 shards of 256 (one per NeuronCore);
J-sharding avoids replicating the 32MB weights tensor.

Precision: the HBM side runs in bf16, halving traffic to 20MB per
core: input1 2MB + input2 shard 8MB + weights shard 2MB read, out
shard 8MB written.  The K-reduction and GEMM accumulate in f32 (DVE
reduce -> f32, matmul -> PSUM f32); bf16 rounding enters only on the
stored operands and the final output (rel-l2 vs the f32 reference
~4e-3, well under the 2e-2 gate).  The host casts/gathers.

Host staging puts every per-core DRAM array in the exact layout the
kernel consumes, so all DMAs move 2-16KB contiguous runs/partition:
  x1t[q*64+i, n*128+p]       = input1[n*256 + 2p + q, i]
  wd[h*64+i, j''*64+k]       = weights[i, jsl(h*128+j''), k]
  x2d[g, p, (s*2+q)*256+j]   = input2[(g*8+s)*256 + 2p + q, jsl(j)]
  (out is produced in the x2d layout and un-permuted on the host)

Per-core kernel:
  - wd loaded in 8 chunks of 256KB alternating between the two HWDGE
    rings (scalar/ACT gets even chunks, sync/SP odd), each reduced
    over k by DVE as it lands -> w2h (128, 128) f32, partition h*64+i
    holds w2[i, h-half cols].  Pipelining the reduce under the loads
    keeps the w2 chain off the critical path (v1 lost ~12us here).
  - two selection-matrix matmuls (sel_h[h*64+i, q*64+i] = 1) fan
    w2h out to pdup[q*64+i, h*128+j''] = w2[i, j'] (PSUM f32); one
    DVE copy casts to w2dup (128, 256) bf16 so each PE row-group q
    holds the full reduced weight matrix.
  - x1t loads on the sync ring behind the odd w chunks (0.5MB head
    chunk covering groups 0-1 first, then the 1.5MB rest).
  - 8 groups of 8 super-tiles (256 rows each): per group one 1MB x2
    load (scalar ring); per super-tile PAIR four matmuls (q=0/q=1 on
    disjoint PE row-groups run concurrently; the two u-steps share a
    PSUM bank half each: col q*512 + u*256) into a 2-bank PSUM tile,
    then ONE DVE multiply for the pair (1024 elems/partition -- half
    the per-instruction read-write-bubble overhead of per-super-tile
    muls); two 512KB half-group stores (sync ring).
"""

import numpy as np

B, I, J, K = 16384, 64, 2048, 64
NCORES = 8
JS = J // NCORES          # 256 columns per core
NSUP = B // 256           # 64 super-tiles of 256 rows
GROUP = 8                 # super-tiles per DMA group
NG = NSUP // GROUP        # 8 groups
NPAIR = GROUP // 2        # super-tile pairs per group
XBUFS = 4                 # xtile buffer depth
OBUFS = 3                 # otile buffer depth
NWCHUNK = 8               # weights load chunks
GFREE = GROUP * 2 * JS    # 4096 elems per partition per group

_CACHE = {}


def _build_nc():
    from contextlib import ExitStack

    import concourse.mybir as mybir
    import concourse.tile as tile
    from concourse import bacc

    f32 = mybir.dt.float32
    bf16 = mybir.dt.bfloat16
    nc = bacc.Bacc()

    x1 = nc.dram_tensor("input1", [128, NSUP * 128], bf16, kind="ExternalInput")
    x2 = nc.dram_tensor("input2", [NG, 128, GFREE], bf16, kind="ExternalInput")
    w = nc.dram_tensor("weights", [128, (JS // 2) * K], bf16, kind="ExternalInput")
    out = nc.dram_tensor("out", [NG, 128, GFREE], bf16, kind="ExternalOutput")

    with tile.TileContext(nc) as tc, ExitStack() as ctx:
        const_pool = ctx.enter_context(tc.tile_pool(name="const", bufs=1))
        wc_pool = ctx.enter_context(tc.tile_pool(name="wc", bufs=1))
        x_pool = ctx.enter_context(tc.tile_pool(name="xin", bufs=XBUFS))
        o_pool = ctx.enter_context(tc.tile_pool(name="oout", bufs=OBUFS))
        yb_pool = ctx.enter_context(tc.tile_pool(name="yb", bufs=3))
        ps_pool = ctx.enter_context(tc.tile_pool(name="ps", bufs=3, space="PSUM"))
        tr_pool = ctx.enter_context(tc.tile_pool(name="tr", bufs=1, space="PSUM"))

        # selection masks: sel_h[h*64+i, q*64+i] = 1, else 0
        # (sel_h.T @ w2h)[q*64+i, j''] = w2h[h*64+i, j'']
        sel = []
        for h in range(2):
            sh = const_pool.tile([128, 128], f32, name=f"sel{h}")
            sel.append(sh)
            nc.gpsimd.memset(sh[:], 0.0)
            for q in range(2):
                # fill where p - 64h - m' == 0 over the (128, 64) column block
                nc.gpsimd.affine_select(
                    out=sh[:, q * 64 : (q + 1) * 64],
                    in_=sh[:, q * 64 : (q + 1) * 64],
                    compare_op=mybir.AluOpType.not_equal,
                    fill=1.0,
                    base=-64 * h,
                    pattern=[[-1, 64]],
                    channel_multiplier=1,
                )

        # ---- weights: 8 chunk loads front-loaded on both rings, then
        # pipelined DVE reduce (chunks 0-3 head the scalar ring, 4-7 the
        # sync ring, so w gets both rings' full bandwidth first) ----
        w2h = const_pool.tile([128, JS // 2], f32)  # (128, 128), part h*64+i
        wcsz = (JS // 2) * K // NWCHUNK  # 1024 elems per partition per chunk
        jcs = (JS // 2) // NWCHUNK       # 16 w2h columns per chunk
        wchunks = []
        for c in range(NWCHUNK):
            wchunk = wc_pool.tile([128, wcsz], bf16, name=f"wchunk{c}", tag=f"wc{c}")
            wchunks.append(wchunk)
            # interleave ring order with landing order: c even -> scalar,
            # c odd -> sync, so reduces can run in emission order c=0..7
            eng = nc.scalar if c % 2 == 0 else nc.sync
            eng.dma_start(out=wchunk[:], in_=w[:, c * wcsz : (c + 1) * wcsz])
        for c in range(NWCHUNK):
            nc.vector.tensor_reduce(
                out=w2h[:, c * jcs : (c + 1) * jcs],
                in_=wchunks[c][:].rearrange("p (j k) -> p j k", k=K),
                axis=mybir.AxisListType.X,
                op=mybir.AluOpType.add,
            )

        # ---- x1t load on sync ring (head chunk first for groups 0-1) ----
        x1T = const_pool.tile([128, NSUP * 128], bf16)
        nc.sync.dma_start(out=x1T[:, 0:2048], in_=x1[:, 0:2048])
        nc.sync.dma_start(out=x1T[:, 2048:], in_=x1[:, 2048:])

        # ---- x2 group prefetch (scalar ring) ----
        xtiles = []

        def load(g):
            assert len(xtiles) == g
            xt = x_pool.tile([128, GFREE], bf16, name=f"xt{g}", tag="xt")
            xtiles.append(xt)
            nc.scalar.dma_start(out=xt[:], in_=x2[g])

        for g in range(XBUFS):
            load(g)

        # ---- de-interleave + duplicate w2 via selection matmuls ----
        # pdup[q*64+i, h*128+j''] = w2h[h*64+i, j'']
        pdup = tr_pool.tile([128, JS], f32)
        for h in range(2):
            nc.tensor.matmul(
                pdup[:, h * 128 : (h + 1) * 128],
                lhsT=sel[h][:],
                rhs=w2h[:],
                start=True,
                stop=True,
            )
        w2dup = const_pool.tile([128, JS], bf16)
        nc.vector.tensor_copy(out=w2dup[:], in_=pdup[:])

        # ---- main loop ----
        # Per super-tile pair: 4 matmuls -> PSUM f32; ScalarE (ACT, idle
        # otherwise, has the PSUM port) casts the pair to SBUF bf16; DVE
        # multiplies bf16*bf16 -> bf16 at its 2x 16-bit rate.
        def process(g):
            xt = xtiles[g]
            ot = o_pool.tile([128, GFREE], bf16, name=f"ot{g}", tag="ot")
            for mm in range(NPAIR):
                # 4 matmuls into a 2-bank PSUM tile: col q*512 + u*256;
                # the q pair runs concurrently on disjoint PE row-groups
                # into different banks, u-steps fill the bank halves.
                pt = ps_pool.tile([128, 4 * JS], f32)  # (128, 1024)
                for u in range(2):
                    n = (g * NPAIR + mm) * 2 + u
                    for q in range(2):
                        nc.tensor.matmul(
                            pt[:, q * 512 + u * JS : q * 512 + (u + 1) * JS],
                            lhsT=x1T[
                                q * 64 : (q + 1) * 64, n * 128 : (n + 1) * 128
                            ],
                            rhs=w2dup[q * 64 : (q + 1) * 64, :],
                            start=True,
                            stop=True,
                        )
                yb = yb_pool.tile([128, 4 * JS], bf16, name="yb", tag="yb")
                nc.scalar.copy(yb[:], pt[:])
                nc.vector.tensor_mul(
                    ot[:, mm * 1024 : (mm + 1) * 1024].rearrange(
                        "p (u q j) -> p q u j", u=2, q=2
                    ),
                    yb[:].rearrange("p (q u j) -> p q u j", q=2, u=2),
                    xt[:, mm * 1024 : (mm + 1) * 1024].rearrange(
                        "p (u q j) -> p q u j", u=2, q=2
                    ),
                )
                if mm % 2 == 1:
                    half = mm // 2
                    nc.sync.dma_start(
                        out=out[g][:, half * 2048 : (half + 1) * 2048],
                        in_=ot[:, half * 2048 : (half + 1) * 2048],
                    )
            if g + XBUFS < NG:
                load(g + XBUFS)

        for g in range(NG):
            process(g)

    nc.compile()
    return nc


def _get_nc():
    if "nc" not in _CACHE:
        _CACHE["nc"] = _build_nc()
    return _CACHE["nc"]


def _make_in_maps(input1, input2, weights):
    import ml_dtypes

    BF = ml_dtypes.bfloat16
    input1 = np.asarray(input1, dtype=np.float32)
    input2 = np.asarray(input2, dtype=np.float32)
    weights = np.asarray(weights, dtype=np.float32)

    # x1t[q*64+i, n*128+p] = input1[n*256 + 2p + q, i]
    x1t = (
        input1.reshape(NSUP, 128, 2, I)
        .transpose(2, 3, 0, 1)
        .reshape(128, NSUP * 128)
        .astype(BF)
    )

    in_maps = []
    for c in range(NCORES):
        sl = slice(c * JS, (c + 1) * JS)
        # wd[h*64+i, j''*64+k] = weights[i, c*JS + h*128 + j'', k]
        wd = (
            weights[:, sl, :]
            .reshape(I, 2, 128, K)
            .transpose(1, 0, 2, 3)
            .reshape(128, 128 * K)
            .astype(BF)
        )
        # x2d[g, p, (s*2+q)*256+j] = input2[(g*8+s)*256 + 2p + q, sl][j]
        x2d = (
            input2[:, sl]
            .reshape(NG, GROUP, 128, 2, JS)
            .transpose(0, 2, 1, 3, 4)
            .reshape(NG, 128, GFREE)
            .astype(BF)
        )
        in_maps.append({"input1": x1t, "input2": x2d, "weights": wd})
    return in_maps


def run(input1, input2, weights, trace=False, **spmd_kwargs):
    from concourse.bass_utils import run_bass_kernel_spmd

    nc = _get_nc()
    in_maps = _make_in_maps(input1, input2, weights)
    res = run_bass_kernel_spmd(
        nc, in_maps, core_ids=list(range(NCORES)), trace=trace, **spmd_kwargs
    )
    outs = []
    for c in range(NCORES):
        o = np.asarray(res.results[c]["out"])  # (NG, 128, GFREE) bf16
        outs.append(
            o.reshape(NG, 128, GROUP, 2, JS)
            .transpose(0, 2, 1, 3, 4)
            .reshape(B, JS)
        )
    full = np.concatenate(outs, axis=1).astype(np.float32)
    return full, res


def kernel(input1, input2, weights):
    full, _ = run(input1, input2, weights, trace=False)
    return full
